# revision 1
# baseline (speedup 1.0000x reference)
"""Multi-head attention (B=2, L=2048, C=1024, H=16, D=64) on 8 trn2 NeuronCores.

Sharding: core c -> (batch b = c//4, head-group g = c%4, 4 heads per group).
Tensor-parallel over heads: W_q/W_k/W_v column-sliced per group, W_o
row-sliced; each core returns a partial output projection for its batch and
the host sums the 4 group partials (the all-reduce of the hint, done on host
since the kernel contract is full-in/full-out).

Per-core device pipeline (all matmuls bf16 with fp32 PSUM accumulation):
  A) Qt, Kt = (x @ W)^T in [channels, L] layout via W^T-stationary matmuls;
     V in natural [L, channels] layout with 4 interleaved ones-columns
     ([V_h | 1] per head) for the softmax row-sum trick.
  B) S^T tiles (Lk on partitions) per (lq-block, head); exp on ScalarE
     reading wide PSUM APs, writing bf16 P^T tiles (scale=1/sqrt(D) folded
     into the activation's free affine).
  C) Yext^T = [V_h | 1]^T @ P^T  (M=65: row 64 = softmax denominator),
     then row-normalize via DVE reciprocal + GPSIMD partition-broadcast.
  D) out_partial = Ybar @ Wo_g, fp32, DMA'd out.

b_k is dropped (adds a per-query constant to logits -> softmax invariant);
b_v is folded into the host-side constant (b_v @ W_o + b_o) since sum(P)=1.
"""

import os
import numpy as np
import ml_dtypes

import concourse.bass as bass
import concourse.mybir as mybir
from concourse import bacc
from concourse.tile import TileContext
from concourse.bass_utils import run_bass_kernel_spmd

BF16 = ml_dtypes.bfloat16
BF = mybir.dt.bfloat16
F32 = mybir.dt.float32

B, L, C = 2, 2048, 1024
H, D = 16, 64
NG = 4            # head-groups (cores per batch)
HPG = 4           # heads per group
GC = HPG * D      # 256 channels per group
KC = C // 128     # 8 contraction chunks for projections
LC = L // 128     # 16 L-chunks
NBLK = 4          # lq blocks of 512
BLKW = 512
# lk-chunk pieces per (blk, head): psum tiles of 3 chunks (1536 cols) + 1
PIECES = [(0, 3), (3, 3), (6, 3), (9, 3), (12, 3), (15, 1)]
VW = HPG * (D + 1)  # 260: interleaved [V_h | ones] columns

_CACHE = {}


def _build(debug_dump=False):
    nc = bacc.Bacc("TRN2", target_bir_lowering=False, debug=False, num_devices=8)
    dbg = {}
    if debug_dump:
        dbg["qt0"] = nc.declare_dram_parameter("d_qt0", [128, L], BF, isOutput=True)
        dbg["kt0"] = nc.declare_dram_parameter("d_kt0", [128, L], BF, isOutput=True)
        dbg["v0"] = nc.declare_dram_parameter("d_v0", [128, VW], BF, isOutput=True)
        dbg["pt"] = nc.declare_dram_parameter("d_pt", [128, 3 * BLKW], BF, isOutput=True)
        dbg["yext"] = nc.declare_dram_parameter("d_yext", [D + 1, BLKW], F32, isOutput=True)
        dbg["rbc"] = nc.declare_dram_parameter("d_rbc", [64, BLKW], F32, isOutput=True)

    xq = nc.declare_dram_parameter("xqT", [C, L], BF, isOutput=False)
    xk = nc.declare_dram_parameter("xkT", [C, L], BF, isOutput=False)
    xv = nc.declare_dram_parameter("xvT", [C, L], BF, isOutput=False)
    wq = nc.declare_dram_parameter("wq", [128, KC * GC], BF, isOutput=False)
    wk = nc.declare_dram_parameter("wk", [128, KC * GC], BF, isOutput=False)
    wv = nc.declare_dram_parameter("wv", [128, KC * VW], BF, isOutput=False)
    wo = nc.declare_dram_parameter("wo", [128, 2 * C], BF, isOutput=False)
    bq = nc.declare_dram_parameter("bq", [GC, 1], F32, isOutput=False)
    out = nc.declare_dram_parameter("out", [L, C], F32, isOutput=True)

    with TileContext(nc) as tc:
        with (
            tc.tile_pool(name="w", bufs=1) as wpool,
            tc.tile_pool(name="x", bufs=3) as xpool,
            tc.tile_pool(name="xv", bufs=1) as xvpool,
            tc.tile_pool(name="qk", bufs=1) as qkpool,
            tc.tile_pool(name="vsb", bufs=1) as vpool,
            tc.tile_pool(name="pt", bufs=22) as ptpool,
            tc.tile_pool(name="sm", bufs=3) as smpool,
            tc.tile_pool(name="ob", bufs=4) as opool,
            tc.tile_pool(name="ps", bufs=2, space="PSUM") as spsum,
            tc.tile_pool(name="py", bufs=2, space="PSUM") as ypsum,
        ):
            # ---- weights / constants into SBUF ----
            wq_sb = wpool.tile([128, KC * GC], BF, tag="wq")
            nc.sync.dma_start(wq_sb[:], wq[:])
            wk_sb = wpool.tile([128, KC * GC], BF, tag="wk")
            nc.gpsimd.dma_start(wk_sb[:], wk[:])
            wv_sb = wpool.tile([128, KC * VW], BF, tag="wv")
            wo_sb = wpool.tile([128, 2 * C], BF, tag="wo")
            bq_sb = []
            for oc in range(2):
                t = wpool.tile([128, 1], F32, tag=f"bq{oc}")
                nc.gpsimd.dma_start(t[:], bq[oc * 128:(oc + 1) * 128, :])
                bq_sb.append(t)
            # ones-pattern: +1.0 at the interleaved ones-columns (65h+64)
            vpat = wpool.tile([128, VW], F32, tag="vpat")
            nc.vector.memset(vpat[:], 0.0)
            for h in range(HPG):
                nc.vector.memset(vpat[:, h * (D + 1) + D: h * (D + 1) + D + 1], 1.0)

            # ---- A2: Q/K projections -> transposed [GC, L] layout ----
            qt_sb = [
                qkpool.tile([128, L], BF, tag=f"qt{oc}", name=f"qt{oc}")
                for oc in range(2)
            ]
            kt_sb = [
                qkpool.tile([128, L], BF, tag=f"kt{oc}", name=f"kt{oc}")
                for oc in range(2)
            ]
            for (xdram, wsb, dst, bias) in (
                (xq, wq_sb, qt_sb, bq_sb),
                (xk, wk_sb, kt_sb, None),
            ):
                # both oc-chunks accumulate in flight: 2x [128,1536] (tag s)
                # + 2x [128,512] (tag y) psum tiles; x streams through once.
                ps_a = [spsum.tile([128, 3 * BLKW], F32, tag="s", name=f"psa{i}") for i in range(2)]
                ps_b = [ypsum.tile([128, BLKW], F32, tag="y", name=f"psb{i}") for i in range(2)]
                for kc in range(KC):
                    xt = xpool.tile([128, L], BF, tag=f"x{kc % 2}")
                    nc.sync.dma_start(xt[:], xdram[kc * 128:(kc + 1) * 128, :])
                    for oc in range(2):
                        lhs = wsb[:, kc * GC + oc * 128: kc * GC + (oc + 1) * 128]
                        for lqb in range(NBLK):
                            tgt = (
                                ps_a[oc][:, lqb * BLKW:(lqb + 1) * BLKW]
                                if lqb < 3 else ps_b[oc][:]
                            )
                            nc.tensor.matmul(
                                tgt,
                                lhs,
                                xt[:, lqb * BLKW:(lqb + 1) * BLKW],
                                start=(kc == 0),
                                stop=(kc == KC - 1),
                            )
                # wide copies on DVE, narrow ones on the (startup-idle)
                # ScalarE so the PSUM->SBUF drain doesn't serialize on one
                # engine right before the attention phase can begin.
                Ident = mybir.ActivationFunctionType.Identity
                for oc in range(2):
                    if bias is not None:
                        nc.vector.tensor_scalar_add(
                            dst[oc][:, 0:3 * BLKW], ps_a[oc][:], bias[oc][:]
                        )
                        nc.scalar.activation(
                            dst[oc][:, 3 * BLKW:L], ps_b[oc][:], Ident,
                            bias=bias[oc][:],
                        )
                    else:
                        nc.vector.tensor_copy(dst[oc][:, 0:3 * BLKW], ps_a[oc][:])
                        nc.scalar.copy(dst[oc][:, 3 * BLKW:L], ps_b[oc][:])

            if debug_dump:
                nc.sync.dma_start(dbg["qt0"][:], qt_sb[0][:])
                nc.sync.dma_start(dbg["kt0"][:], kt_sb[0][:])

            # ---- V projection (emitted inside main loop as filler) ----
            def emit_V():
                nc.gpsimd.dma_start(wv_sb[:], wv[:])
                nc.gpsimd.dma_start(wo_sb[:], wo[:])
                xv_sb = []
                for kc in range(KC):
                    t = xvpool.tile([128, L], BF, tag=f"xv{kc % 8}", name="xvt")
                    nc.sync.dma_start(t[:], xv[kc * 128:(kc + 1) * 128, :])
                    xv_sb.append(t)
                for lc in range(LC):
                    vps = ypsum.tile([128, BLKW], F32, tag="y", name="vps")
                    for kc in range(KC):
                        nc.tensor.matmul(
                            vps[:, 0:VW],
                            xv_sb[kc][:, lc * 128:(lc + 1) * 128],
                            wv_sb[:, kc * VW:(kc + 1) * VW],
                            start=(kc == 0),
                            stop=(kc == KC - 1),
                        )
                    vt = vpool.tile([128, VW], BF, tag=f"v{lc}", name="vt")
                    nc.vector.tensor_add(vt[:], vps[:, 0:VW], vpat[:])
                    v_sb.append(vt)
                if debug_dump:
                    nc.sync.dma_start(dbg["v0"][:], v_sb[0][:])
            # ---- main loop: attention + output projection ----
            # Pair-pipelined emission: for pair index p, emit B(p) (S^T+exp)
            # BEFORE C(p-1) (PV+normalize), so S^T production outranks PV in
            # scheduler priority and the ScalarE exp pipeline never starves
            # behind a PV burst. D(blk) trails its last pair by one slot.
            ExpF = mybir.ActivationFunctionType.Exp
            scale = 1.0 / float(np.sqrt(D))
            ybar = [
                qkpool.tile([128, L], BF, tag=f"yb{oc}", name=f"yb{oc}")
                for oc in range(2)
            ]
            v_sb = []
            ptmaps = {}

            def emit_B(p):
                blk, hp = divmod(p, 2)
                lq0 = blk * BLKW
                units = [(hh, lk) for lk in range(LC) for hh in range(2)]
                ptloc = {}
                for u0 in range(0, len(units), 3):
                    grp = units[u0:u0 + 3]
                    w = len(grp) * BLKW
                    ps = spsum.tile([128, 3 * BLKW], F32, tag="s", name="pss")
                    for i, (hh, lk) in enumerate(grp):
                        h = 2 * hp + hh
                        oc = h // 2
                        r0 = (h % 2) * 64
                        nc.tensor.matmul(
                            ps[:, i * BLKW:(i + 1) * BLKW],
                            kt_sb[oc][r0:r0 + 64, lk * 128:(lk + 1) * 128],
                            qt_sb[oc][r0:r0 + 64, lq0:lq0 + BLKW],
                            start=True,
                            stop=True,
                        )
                    pt = ptpool.tile([128, 3 * BLKW], BF, tag="pt")
                    nc.scalar.activation(pt[:, 0:w], ps[:, 0:w], ExpF, scale=scale)
                    for i, (hh, lk) in enumerate(grp):
                        ptloc[(hh, lk)] = (pt, i * BLKW)
                ptmaps[p] = ptloc

            def emit_C(p, tail=False):
                blk, hp = divmod(p, 2)
                lq0 = blk * BLKW
                ptloc = ptmaps.pop(p)
                yexts = [
                    ypsum.tile([128, BLKW], F32, tag="y", name="yext")
                    for _ in range(2)
                ]
                for lk in range(LC):
                    for hh in range(2):
                        h = 2 * hp + hh
                        pt, col = ptloc[(hh, lk)]
                        nc.tensor.matmul(
                            yexts[hh][0:D + 1, :],
                            v_sb[lk][:, h * (D + 1):(h + 1) * (D + 1)],
                            pt[:, col:col + BLKW],
                            start=(lk == 0),
                            stop=(lk == LC - 1),
                        )
                for hh in range(2):
                    h = 2 * hp + hh
                    yext = yexts[hh]
                    if debug_dump and p == 0 and hh == 0:
                        ydump = smpool.tile([D + 1, BLKW], F32, tag="ydump")
                        nc.vector.tensor_copy(ydump[:], yext[0:D + 1, :])
                        nc.sync.dma_start(dbg["yext"][:], ydump[:])
                    rs = smpool.tile([1, BLKW], F32, tag="rs")
                    if tail:
                        nc.scalar.copy(rs[:], yext[D:D + 1, :])
                    else:
                        nc.vector.tensor_copy(rs[:], yext[D:D + 1, :])
                    rr = smpool.tile([1, BLKW], F32, tag="rr")
                    nc.vector.reciprocal_approx_fast(rr[:], rs[:])
                    rbc = smpool.tile([64, BLKW], F32, tag="rbc")
                    nc.gpsimd.partition_broadcast(rbc[:], rr[:])
                    if debug_dump and p == 0 and hh == 0:
                        nc.sync.dma_start(dbg["rbc"][:], rbc[:])
                    oc = h // 2
                    r0 = (h % 2) * 64
                    nc.vector.tensor_mul(
                        ybar[oc][r0:r0 + 64, lq0:lq0 + BLKW],
                        yext[0:D, :],
                        rbc[:],
                    )

            def emit_D(blk):
                for lc in range(4 * blk, 4 * blk + 4):
                    osb = opool.tile([128, C], F32, tag="osb")
                    for cc in range(2):
                        po = ypsum.tile([128, BLKW], F32, tag="y")
                        for kc2 in range(2):
                            nc.tensor.matmul(
                                po[:],
                                ybar[kc2][:, lc * 128:(lc + 1) * 128],
                                wo_sb[:, kc2 * C + cc * BLKW: kc2 * C + (cc + 1) * BLKW],
                                start=(kc2 == 0),
                                stop=(kc2 == 1),
                            )
                        nc.vector.tensor_copy(
                            osb[:, cc * BLKW:(cc + 1) * BLKW], po[:]
                        )
                    eng = nc.sync if lc % 2 == 0 else nc.gpsimd
                    eng.dma_start(out[lc * 128:(lc + 1) * 128, :], osb[:])

            emit_B(0)
            emit_B(1)
            emit_V()   # V projection: PE filler under the first S/exp phases
            emit_C(0)
            for p in range(2, 8):
                emit_B(p)
                emit_C(p - 1)
                if p in (3, 5, 7):
                    # D has a full block of slack: deprioritize it so it only
                    # fills PE gaps instead of preempting S^T production.
                    with tc.high_priority(offset=-400):
                        emit_D((p - 3) // 2)
            emit_C(7, tail=True)
            emit_D(3)

    nc.compile()
    return nc


def _get_nc():
    if "nc" not in _CACHE:
        _CACHE["nc"] = _build()
    return _CACHE["nc"]


def _prep_in_maps(q, k, v, Wq, bq, Wk, bv_unused, Wv, Wo):
    del bv_unused
    xT = {}
    for b in range(B):
        xT[("q", b)] = np.ascontiguousarray(q[b].T).astype(BF16)
        xT[("k", b)] = np.ascontiguousarray(k[b].T).astype(BF16)
        xT[("v", b)] = np.ascontiguousarray(v[b].T).astype(BF16)
    in_maps = []
    for c in range(8):
        b, g = c // NG, c % NG
        wv_g = np.zeros((C, VW), dtype=BF16)
        for h in range(HPG):
            wv_g[:, h * (D + 1):h * (D + 1) + D] = Wv[
                :, g * GC + h * D: g * GC + (h + 1) * D
            ].astype(BF16)
        def tile_w(w):
            # [C_or_GC rows, n cols] -> [128, KC_chunks * n]: chunk kc holds
            # rows kc*128..kc*128+128 side by side
            r, n = w.shape
            return np.ascontiguousarray(
                w.reshape(r // 128, 128, n).transpose(1, 0, 2).reshape(128, -1)
            )
        in_maps.append({
            "xqT": xT[("q", b)],
            "xkT": xT[("k", b)],
            "xvT": xT[("v", b)],
            "wq": tile_w(Wq[:, g * GC:(g + 1) * GC].astype(BF16)),
            "wk": tile_w(Wk[:, g * GC:(g + 1) * GC].astype(BF16)),
            "wv": tile_w(wv_g),
            "wo": tile_w(Wo[g * GC:(g + 1) * GC, :].astype(BF16)),
            "bq": bq[g * GC:(g + 1) * GC].reshape(GC, 1).astype(np.float32),
        })
    return in_maps


def kernel(q, k, v, Wq, bq, Wk, bk, Wv, bv, Wo, bo):
    q = np.asarray(q, dtype=np.float32)
    k = np.asarray(k, dtype=np.float32)
    v = np.asarray(v, dtype=np.float32)
    Wq = np.asarray(Wq, dtype=np.float32)
    bq = np.asarray(bq, dtype=np.float32)
    Wk = np.asarray(Wk, dtype=np.float32)
    Wv = np.asarray(Wv, dtype=np.float32)
    bv = np.asarray(bv, dtype=np.float32)
    Wo = np.asarray(Wo, dtype=np.float32)
    bo = np.asarray(bo, dtype=np.float32)

    nc = _get_nc()
    in_maps = _prep_in_maps(q, k, v, Wq, bq, Wk, None, Wv, Wo)

    trace = bool(int(os.environ.get("BASS_KERNEL_PROFILE", "0")))
    kwargs = {}
    if trace:
        try:
            from ntff_hook import install as _install_hook
            _install_hook()
        except Exception:
            pass
        kwargs = {"trace": True}
        td = os.environ.get("BASS_KERNEL_TRACE_DIR")
        if td:
            kwargs["tmpdir"] = td
    res = run_bass_kernel_spmd(nc, in_maps, core_ids=list(range(8)), **kwargs)
    _CACHE["last_exec_time_ns"] = res.exec_time_ns

    # host gather: sum group partials per batch, add folded bias
    cvec = (bv.astype(np.float64) @ Wo.astype(np.float64) + bo).astype(np.float32)
    full = np.empty((B, L, C), dtype=np.float32)
    for b in range(B):
        acc = res.results[b * NG]["out"].astype(np.float32)
        for g in range(1, NG):
            acc = acc + res.results[b * NG + g]["out"]
        full[b] = acc + cvec[None, :]
    return full



# revision 11
# speedup vs baseline: 1.0558x; 1.0558x over previous
"""Multi-head attention (B=2, L=2048, C=1024, H=16, D=64) on 8 trn2 NeuronCores.

Sharding: core c -> (batch b = c//4, head-group g = c%4, 4 heads per group).
Tensor-parallel over heads: W_q/W_k/W_v column-sliced per group, W_o
row-sliced; each core returns a bf16 partial output projection for its batch
and the host sums the 4 group partials in fp32 (the all-reduce of the hint,
done on host since the kernel contract is full-in/full-out).

Pipeline (all matmuls bf16 with fp32 PSUM accumulation). The kernel is
ScalarE-bound (exp of 16.8M logits/core at 1 elem/cycle/lane), so the
schedule exists to (a) start the first exp as early as possible and (b) keep
the exp stream dense:

  - Inputs are DMA'd in lq/lk-block-major "slabs" [128, 8*512] so K^T block 0
    and Q^T block 0 are ready ~18us in; attention starts while the remaining
    projections stream in as PE filler.
  - S^T tiles per (lq-block, head-pair): K=64 matmuls auto-row-tiled in the
    PE array (heads at partitions 0:64 / 64:128 run concurrently);
    exp on ScalarE over [128,1536] PSUM tiles (scale=1/sqrt(D) folded in).
  - PV: Yext^T = [V_h | 1]^T @ P^T (row 64 = softmax denominator),
    head-sequential on a single PSUM bank; yext is copied to SBUF so the
    bank frees early; normalize via DVE reciprocal + GPSIMD broadcast.
  - Output projection per 128-row chunk on a second single-bank PSUM ring,
    drained to bf16 and DMA'd out.

b_k is dropped (softmax-invariant); b_v is folded into the host-side
constant (b_v @ W_o + b_o) since sum(P)=1.
"""

import os
import numpy as np
import ml_dtypes

import concourse.bass as bass
import concourse.mybir as mybir
from concourse import bacc
from concourse.tile import TileContext
from concourse.bass_utils import run_bass_kernel_spmd

BF16 = ml_dtypes.bfloat16
BF = mybir.dt.bfloat16
F32 = mybir.dt.float32

B, L, C = 2, 2048, 1024
H, D = 16, 64
NG = 4            # head-groups (cores per batch)
HPG = 4           # heads per group
GC = HPG * D      # 256 channels per group
KC = C // 128     # 8 contraction chunks for projections
LC = L // 128     # 16 L-chunks
NBLK = 4          # l blocks of 512
BLKW = 512
SLABW = KC * BLKW  # 4096 cols per slab
VW = HPG * (D + 1)  # 260: interleaved [V_h | ones] columns

_CACHE = {}


def _build(debug_dump=False):
    nc = bacc.Bacc("TRN2", target_bir_lowering=False, debug=False, num_devices=8)
    dbg = {}
    if debug_dump:
        dbg["qt0"] = nc.declare_dram_parameter("d_qt0", [128, L], BF, isOutput=True)
        dbg["kt0"] = nc.declare_dram_parameter("d_kt0", [128, L], BF, isOutput=True)
        dbg["v0"] = nc.declare_dram_parameter("d_v0", [128, VW], BF, isOutput=True)
        dbg["pt0"] = nc.declare_dram_parameter("d_pt0", [128, 3 * BLKW], BF, isOutput=True)
        dbg["yb0"] = nc.declare_dram_parameter("d_yb0", [128, L], BF, isOutput=True)
        dbg["ys"] = nc.declare_dram_parameter("d_ys", [D + 1, BLKW], F32, isOutput=True)
        dbg["rbc"] = nc.declare_dram_parameter("d_rbc", [64, BLKW], F32, isOutput=True)

    xq = nc.declare_dram_parameter("xq", [128, NBLK * SLABW], BF, isOutput=False)
    xk = nc.declare_dram_parameter("xk", [128, NBLK * SLABW], BF, isOutput=False)
    xv = nc.declare_dram_parameter("xv", [128, NBLK * SLABW], BF, isOutput=False)
    wq = nc.declare_dram_parameter("wq", [128, KC * GC], BF, isOutput=False)
    wk = nc.declare_dram_parameter("wk", [128, KC * GC], BF, isOutput=False)
    wv = nc.declare_dram_parameter("wv", [128, KC * VW], BF, isOutput=False)
    wo = nc.declare_dram_parameter("wo", [128, 2 * C], BF, isOutput=False)
    bq = nc.declare_dram_parameter("bq", [GC, 1], F32, isOutput=False)
    out = nc.declare_dram_parameter("out", [L, C], BF, isOutput=True)

    Ident = mybir.ActivationFunctionType.Identity
    ExpF = mybir.ActivationFunctionType.Exp
    scale = 1.0 / float(np.sqrt(D))

    with TileContext(nc) as tc:
        with (
            tc.tile_pool(name="w", bufs=1) as wpool,
            tc.tile_pool(name="xs", bufs=1) as xspool,
            tc.tile_pool(name="qk", bufs=1) as qkpool,
            tc.tile_pool(name="vsb", bufs=1) as vpool,
            tc.tile_pool(name="pt", bufs=24) as ptpool,
            tc.tile_pool(name="sm", bufs=2) as smpool,
            tc.tile_pool(name="ob", bufs=3) as opool,
            tc.tile_pool(name="ps", bufs=2, space="PSUM") as spsum,
            tc.tile_pool(name="yx", bufs=1, space="PSUM") as yxpsum,
            tc.tile_pool(name="fp", bufs=1, space="PSUM") as fpsum,
        ):
            # ---- weights / constants (gpsimd DMA queue, in order) ----
            wk_sb = wpool.tile([128, KC * GC], BF, tag="wk")
            nc.gpsimd.dma_start(wk_sb[:], wk[:])
            wq_sb = wpool.tile([128, KC * GC], BF, tag="wq")
            nc.gpsimd.dma_start(wq_sb[:], wq[:])
            bq_sb = []
            for oc in range(2):
                t = wpool.tile([128, 1], F32, tag=f"bq{oc}")
                nc.gpsimd.dma_start(t[:], bq[oc * 128:(oc + 1) * 128, :])
                bq_sb.append(t)
            wv_sb = wpool.tile([128, KC * VW], BF, tag="wv")
            nc.gpsimd.dma_start(wv_sb[:], wv[:])
            wo_sb = wpool.tile([128, 2 * C], BF, tag="wo")
            nc.gpsimd.dma_start(wo_sb[:], wo[:])
            # ones-pattern: +1.0 at the interleaved ones-columns (65h+64)
            vpat = wpool.tile([128, VW], F32, tag="vpat")
            nc.vector.memset(vpat[:], 0.0)
            for h in range(HPG):
                nc.vector.memset(vpat[:, h * (D + 1) + D: h * (D + 1) + D + 1], 1.0)

            # ---- input slabs stream through a 4-slot ring on the sync queue
            slab_idx = [0]

            def fetch_slab(xdram, b):
                t = xspool.tile([128, SLABW], BF, tag=f"x{slab_idx[0] % 4}",
                                name="xslab")
                slab_idx[0] += 1
                nc.sync.dma_start(t[:], xdram[:, b * SLABW:(b + 1) * SLABW])
                return t

            # ---- projection chains ----
            qt_sb = [qkpool.tile([128, L], BF, tag=f"qt{oc}", name=f"qt{oc}")
                     for oc in range(2)]
            kt_sb = [qkpool.tile([128, L], BF, tag=f"kt{oc}", name=f"kt{oc}")
                     for oc in range(2)]
            ybar = [qkpool.tile([128, L], BF, tag=f"yb{oc}", name=f"yb{oc}")
                    for oc in range(2)]

            def proj_chain(pool, ptag, wsb, slab, oc, dst_tile, b, bias,
                           drain):
                ps = pool.tile([128, BLKW], F32, tag=ptag, name="projps")
                for kc in range(KC):
                    nc.tensor.matmul(
                        ps[:],
                        wsb[:, kc * GC + oc * 128: kc * GC + (oc + 1) * 128],
                        slab[:, kc * BLKW:(kc + 1) * BLKW],
                        start=(kc == 0),
                        stop=(kc == KC - 1),
                    )
                dst = dst_tile[:, b * BLKW:(b + 1) * BLKW]
                if drain == "scalar":
                    if bias is not None:
                        nc.scalar.activation(dst, ps[:], Ident, bias=bias[:])
                    else:
                        nc.scalar.copy(dst, ps[:])
                else:
                    if bias is not None:
                        nc.vector.tensor_scalar_add(dst, ps[:], bias[:])
                    else:
                        nc.vector.tensor_copy(dst, ps[:])

            # ---- attention emitters ----
            v_sb = []
            ptmaps = {}

            def emit_B(p):
                blk, hp = divmod(p, 2)
                lq0 = blk * BLKW
                units = [(hh, lk) for lk in range(LC) for hh in range(2)]
                ptloc = {}
                for u0 in range(0, len(units), 3):
                    grp = units[u0:u0 + 3]
                    w = len(grp) * BLKW
                    ps = spsum.tile([128, 3 * BLKW], F32, tag="s", name="pss")
                    for i, (hh, lk) in enumerate(grp):
                        h = 2 * hp + hh
                        oc = h // 2
                        r0 = (h % 2) * 64
                        nc.tensor.matmul(
                            ps[:, i * BLKW:(i + 1) * BLKW],
                            kt_sb[oc][r0:r0 + 64, lk * 128:(lk + 1) * 128],
                            qt_sb[oc][r0:r0 + 64, lq0:lq0 + BLKW],
                            start=True,
                            stop=True,
                        )
                    pt = ptpool.tile([128, 3 * BLKW], BF, tag="pt")
                    nc.scalar.activation(pt[:, 0:w], ps[:, 0:w], ExpF, scale=scale)
                    if debug_dump and p == 0 and u0 == 0:
                        nc.sync.dma_start(dbg["pt0"][:], pt[:])
                    for i, (hh, lk) in enumerate(grp):
                        ptloc[(hh, lk)] = (pt, i * BLKW)
                ptmaps[p] = ptloc

            def emit_C_head(p, hh, pool):
                # one head's PV chain + normalize; psum bank from `pool`
                blk, hp = divmod(p, 2)
                lq0 = blk * BLKW
                h = 2 * hp + hh
                ptloc = ptmaps[p]
                yext = pool.tile([128, BLKW], F32, tag="y" if pool is yxpsum
                                 else "f", name="yext")
                for lk in range(LC):
                    pt, col = ptloc[(hh, lk)]
                    nc.tensor.matmul(
                        yext[0:D + 1, :],
                        v_sb[lk][:, h * (D + 1):(h + 1) * (D + 1)],
                        pt[:, col:col + BLKW],
                        start=(lk == 0),
                        stop=(lk == LC - 1),
                    )
                # free the bank early: copy to SBUF, then normalize from there
                ysb = smpool.tile([D + 1, BLKW], F32, tag="ysb")
                nc.vector.tensor_copy(ysb[:], yext[0:D + 1, :])
                rs = smpool.tile([1, BLKW], F32, tag="rs")
                nc.vector.tensor_copy(rs[:], ysb[D:D + 1, :])
                rr = smpool.tile([1, BLKW], F32, tag="rr")
                nc.vector.reciprocal_approx_fast(rr[:], rs[:])
                rbc = smpool.tile([64, BLKW], F32, tag="rbc")
                nc.gpsimd.partition_broadcast(rbc[:], rr[:])
                if debug_dump and p == 0 and hh == 0:
                    nc.sync.dma_start(dbg["ys"][:], ysb[:])
                    nc.sync.dma_start(dbg["rbc"][:], rbc[:])
                oc = h // 2
                r0 = (h % 2) * 64
                nc.vector.tensor_mul(
                    ybar[oc][r0:r0 + 64, lq0:lq0 + BLKW],
                    ysb[0:D, :],
                    rbc[:],
                )

            def emit_C(p, pools=(yxpsum, yxpsum)):
                for hh in range(2):
                    emit_C_head(p, hh, pools[hh])
                ptmaps.pop(p)

            def emit_D_piece(lc, pool):
                osb = opool.tile([128, C], BF, tag="osb")
                for cc in range(2):
                    po = pool.tile(
                        [128, 3 * BLKW] if pool is spsum else [128, BLKW],
                        F32, tag="s" if pool is spsum else "f", name="po")
                    for kc2 in range(2):
                        nc.tensor.matmul(
                            po[:, 0:BLKW],
                            ybar[kc2][:, lc * 128:(lc + 1) * 128],
                            wo_sb[:, kc2 * C + cc * BLKW: kc2 * C + (cc + 1) * BLKW],
                            start=(kc2 == 0),
                            stop=(kc2 == 1),
                        )
                    nc.vector.tensor_copy(
                        osb[:, cc * BLKW:(cc + 1) * BLKW], po[:, 0:BLKW])
                nc.gpsimd.dma_start(out[lc * 128:(lc + 1) * 128, :], osb[:])

            # ================= emission schedule =================
            # head: K block0 (both oc in parallel banks), Q block0, start S.
            sk0 = fetch_slab(xk, 0)
            proj_chain(fpsum, "f", wk_sb, sk0, 0, kt_sb[0], 0, None, "vector")
            proj_chain(yxpsum, "y", wk_sb, sk0, 1, kt_sb[1], 0, None, "scalar")
            sq0 = fetch_slab(xq, 0)
            proj_chain(fpsum, "f", wq_sb, sq0, 0, qt_sb[0], 0, bq_sb[0], "vector")
            proj_chain(yxpsum, "y", wq_sb, sq0, 1, qt_sb[1], 0, bq_sb[1], "scalar")

            # remaining K blocks (needed progressively by B(0)'s lk walk);
            # must be emitted before B(0) so the kt writes are recorded
            # ahead of B(0)'s reads (Tile deps are emission-ordered).
            for b in range(1, NBLK):
                sk = fetch_slab(xk, b)
                proj_chain(fpsum, "f", wk_sb, sk, 0, kt_sb[0], b, None,
                           "vector")
                proj_chain(yxpsum, "y", wk_sb, sk, 1, kt_sb[1], b, None,
                           "vector")

            emit_B(0)
            emit_B(1)

            # V projection: slabs + chains on the yx ring (before PV starts)
            with tc.high_priority(offset=-200):
                for b in range(NBLK):
                    sv = fetch_slab(xv, b)
                    for j in range(4):
                        lc = 4 * b + j
                        vps = yxpsum.tile([128, BLKW], F32, tag="y", name="vps")
                        for kc in range(KC):
                            nc.tensor.matmul(
                                vps[:, 0:VW],
                                sv[:, kc * BLKW + j * 128: kc * BLKW + (j + 1) * 128],
                                wv_sb[:, kc * VW:(kc + 1) * VW],
                                start=(kc == 0),
                                stop=(kc == KC - 1),
                            )
                        vt = vpool.tile([128, VW], BF, tag=f"v{lc}", name="vt")
                        nc.vector.tensor_add(vt[:], vps[:, 0:VW], vpat[:])
                        v_sb.append(vt)

                # Q blocks 1-3 on the fast ring (deadlines: S of p2/p4/p6)
                qsl = {}
                for b in range(1, NBLK):
                    qsl[b] = fetch_slab(xq, b)
                for b in range(1, NBLK):
                    proj_chain(fpsum, "f", wq_sb, qsl[b], 0, qt_sb[0], b,
                               bq_sb[0], "vector")
                    proj_chain(fpsum, "f", wq_sb, qsl[b], 1, qt_sb[1], b,
                               bq_sb[1], "vector")

            emit_C(0)

            # D pieces spread across p's: (p -> lc list)
            dplan = {3: [0, 1, 2], 4: [3, 4, 5], 5: [6, 7],
                     6: [8, 9, 10], 7: [11]}
            for p in range(2, 8):
                emit_B(p)
                emit_C(p - 1)
                if p in dplan:
                    with tc.high_priority(offset=-400):
                        for lc in dplan[p]:
                            emit_D_piece(lc, fpsum)

            # tail: p7's two heads on separate banks so they run concurrently
            emit_C(7, pools=(yxpsum, fpsum))
            for i, lc in enumerate((12, 13, 14, 15)):
                emit_D_piece(lc, spsum if i < 3 else fpsum)

            if debug_dump:
                nc.sync.dma_start(dbg["qt0"][:], qt_sb[0][:])
                nc.sync.dma_start(dbg["kt0"][:], kt_sb[0][:])
                nc.sync.dma_start(dbg["v0"][:], v_sb[0][:])
                nc.sync.dma_start(dbg["yb0"][:], ybar[0][:])

    nc.compile()
    return nc


def _get_nc():
    if "nc" not in _CACHE:
        _CACHE["nc"] = _build()
    return _CACHE["nc"]


def _slabify(xT):
    # xT: [C, L] -> [128, NBLK*SLABW] where slab b, chunk kc at
    # cols b*SLABW + kc*512 holds xT[kc*128:(kc+1)*128, b*512:(b+1)*512]
    A = xT.reshape(KC, 128, NBLK, BLKW)          # [kc, r, b, col]
    return np.ascontiguousarray(
        A.transpose(1, 2, 0, 3).reshape(128, NBLK * SLABW))


def _prep_in_maps(q, k, v, Wq, bq, Wk, Wv, Wo):
    xs = {}
    for b in range(B):
        xs[("q", b)] = _slabify(np.ascontiguousarray(q[b].T).astype(BF16))
        xs[("k", b)] = _slabify(np.ascontiguousarray(k[b].T).astype(BF16))
        xs[("v", b)] = _slabify(np.ascontiguousarray(v[b].T).astype(BF16))
    in_maps = []
    for c in range(8):
        b, g = c // NG, c % NG
        wv_g = np.zeros((C, VW), dtype=BF16)
        for h in range(HPG):
            wv_g[:, h * (D + 1):h * (D + 1) + D] = Wv[
                :, g * GC + h * D: g * GC + (h + 1) * D
            ].astype(BF16)

        def tile_w(w):
            # [rows, n cols] -> [128, chunks * n]: chunk kc holds rows
            # kc*128..(kc+1)*128 side by side
            r, n = w.shape
            return np.ascontiguousarray(
                w.reshape(r // 128, 128, n).transpose(1, 0, 2).reshape(128, -1)
            )
        in_maps.append({
            "xq": xs[("q", b)],
            "xk": xs[("k", b)],
            "xv": xs[("v", b)],
            "wq": tile_w(Wq[:, g * GC:(g + 1) * GC].astype(BF16)),
            "wk": tile_w(Wk[:, g * GC:(g + 1) * GC].astype(BF16)),
            "wv": tile_w(wv_g),
            "wo": tile_w(Wo[g * GC:(g + 1) * GC, :].astype(BF16)),
            "bq": bq[g * GC:(g + 1) * GC].reshape(GC, 1).astype(np.float32),
        })
    return in_maps


def kernel(q, k, v, Wq, bq, Wk, bk, Wv, bv, Wo, bo):
    q = np.asarray(q, dtype=np.float32)
    k = np.asarray(k, dtype=np.float32)
    v = np.asarray(v, dtype=np.float32)
    Wq = np.asarray(Wq, dtype=np.float32)
    bq = np.asarray(bq, dtype=np.float32)
    Wk = np.asarray(Wk, dtype=np.float32)
    Wv = np.asarray(Wv, dtype=np.float32)
    bv = np.asarray(bv, dtype=np.float32)
    Wo = np.asarray(Wo, dtype=np.float32)
    bo = np.asarray(bo, dtype=np.float32)

    nc = _get_nc()
    in_maps = _prep_in_maps(q, k, v, Wq, bq, Wk, Wv, Wo)

    trace = bool(int(os.environ.get("BASS_KERNEL_PROFILE", "0")))
    kwargs = {}
    if trace:
        try:
            from ntff_hook import install as _install_hook
            _install_hook()
        except Exception:
            pass
        kwargs = {"trace": True}
        td = os.environ.get("BASS_KERNEL_TRACE_DIR")
        if td:
            kwargs["tmpdir"] = td
    res = run_bass_kernel_spmd(nc, in_maps, core_ids=list(range(8)), **kwargs)
    _CACHE["last_exec_time_ns"] = res.exec_time_ns

    # host gather: sum group partials per batch in fp32, add folded bias
    cvec = (bv.astype(np.float64) @ Wo.astype(np.float64) + bo).astype(np.float32)
    full = np.empty((B, L, C), dtype=np.float32)
    for b in range(B):
        acc = res.results[b * NG]["out"].astype(np.float32)
        for g in range(1, NG):
            acc = acc + res.results[b * NG + g]["out"].astype(np.float32)
        full[b] = acc + cvec[None, :]
    return full
